# revision 1
# baseline (speedup 1.0000x reference)
"""CoreAttention on 8 Trainium2 cores.

Sharding: 32 (batch, head) pairs -> 4 heads per core (cores 0-3: batch 0,
cores 4-7: batch 1). Per core, per head: scores^T = K Q^T via fp32r (TF32)
matmuls in [t, s] orientation, exp on ACT, mask+TF32-rounding fused into one
DVE multiply with the (inverted) mask, P@V and column-sums as fp32r matmuls
accumulating in PSUM, normalization via reciprocal + rank-1 broadcast matmul.
Host side only slices/transposes/inverts-polarity of inputs (layout prep).
"""
import sys, math
import numpy as np

sys.path.insert(0, "/opt/trn_rl_repo")

SQ, B, NH, HN = 2048, 2, 16, 128
NCORES = 8
HPC = 4                      # heads per core
TC = SQ // 128               # 16 t-chunks
SBLK = 512                   # s-block width
NSB = SQ // SBLK             # 4 s-blocks
SCALE = 1.0 / math.sqrt(128.0)   # COEFF / NORM_FACTOR = 1/sqrt(hn)

_CACHE = {}


def _build(repeat=1):
    import concourse.bacc as bacc
    import concourse.tile as tile
    from concourse import mybir

    F32, F32R, U8 = mybir.dt.float32, mybir.dt.float32r, mybir.dt.uint8
    EXP = mybir.ActivationFunctionType.Exp

    nc = bacc.Bacc(None, target_bir_lowering=False)
    qT_d = nc.dram_tensor("qT", [HPC, HN, SQ], F32R, kind="ExternalInput")
    kT_d = nc.dram_tensor("kT", [HPC, HN, SQ], F32R, kind="ExternalInput")
    v_d = nc.dram_tensor("v", [HPC, SQ, HN], F32R, kind="ExternalInput")
    keep_d = nc.dram_tensor("keepT", [SQ, SQ], U8, kind="ExternalInput")
    ctxT_d = nc.dram_tensor("ctxT", [HPC, HN, SQ], F32, kind="ExternalOutput")

    with tile.TileContext(nc) as tc:
        with (
            tc.tile_pool(name="sbkeep", bufs=1) as sbkeep,
            tc.tile_pool(name="const", bufs=1) as const,
            tc.tile_pool(name="sbqkv", bufs=2) as sbqkv,
            tc.tile_pool(name="sbpt", bufs=2) as sbpt,
            tc.tile_pool(name="sbe", bufs=3) as sbe,
            tc.tile_pool(name="sbmisc", bufs=2) as sbmisc,
            tc.tile_pool(name="pst", bufs=2, space="PSUM") as pst,
            tc.tile_pool(name="psc", bufs=2, space="PSUM") as psc,
            tc.tile_pool(name="pss", bufs=1, space="PSUM") as pss,
            tc.tile_pool(name="psr", bufs=1, space="PSUM") as psr,
        ):
            keep_t = sbkeep.tile([128, TC, SQ], U8, tag="keep")
            nc.sync.dma_start(out=keep_t[:],
                              in_=keep_d.rearrange("(c p) s -> p c s", p=128))

            ones32 = const.tile([128, 1], F32, tag="o32")
            ones1_32 = const.tile([1, 128], F32, tag="o132")
            nc.vector.memset(ones32[:], 1.0)
            nc.vector.memset(ones1_32[:], 1.0)
            ones_r = const.tile([128, 1], F32R, tag="or")
            ones1_r = const.tile([1, 128], F32R, tag="o1r")
            nc.vector.tensor_copy(ones_r[:], ones32[:])
            nc.vector.tensor_copy(ones1_r[:], ones1_32[:])

            def body(_iv=None):
                for h in range(HPC):
                    qT_t = sbqkv.tile([128, SQ], F32R, tag="qT")
                    kT_t = sbqkv.tile([128, SQ], F32R, tag="kT")
                    v_t = sbqkv.tile([128, TC, HN], F32R, tag="v")
                    nc.sync.dma_start(out=qT_t[:], in_=qT_d[h])
                    nc.sync.dma_start(out=kT_t[:], in_=kT_d[h])
                    nc.sync.dma_start(out=v_t[:],
                                      in_=v_d[h].rearrange("(c p) d -> p c d", p=128))
                    for sb in range(NSB):
                        s0 = sb * SBLK
                        pt = sbpt.tile([128, TC, SBLK], F32R, tag="pt")
                        for g in range(TC // 2):
                            st = pst.tile([128, 2, SBLK], F32, tag="st")
                            for j in range(2):
                                ti = 2 * g + j
                                nc.tensor.matmul(
                                    st[:, j, :],
                                    kT_t[:, 128 * ti:128 * (ti + 1)],
                                    qT_t[:, s0:s0 + SBLK],
                                    start=True, stop=True)
                            e32 = sbe.tile([128, 2, SBLK], F32, tag="e")
                            nc.scalar.activation(e32[:], st[:], EXP, scale=SCALE)
                            nc.vector.tensor_mul(
                                pt[:, 2 * g:2 * g + 2, :], e32[:],
                                keep_t[:, 2 * g:2 * g + 2, s0:s0 + SBLK])
                        ctx_p = psc.tile([128, SBLK], F32, tag="ctx")
                        sums_p = pss.tile([1, SBLK], F32, tag="sums")
                        for ti in range(TC):
                            nc.tensor.matmul(ctx_p[:], v_t[:, ti, :], pt[:, ti, :],
                                             start=(ti == 0), stop=(ti == TC - 1))
                        for ti in range(TC):
                            nc.tensor.matmul(sums_p[:], ones_r[:], pt[:, ti, :],
                                             start=(ti == 0), stop=(ti == TC - 1))
                        recip = sbmisc.tile([1, SBLK], F32, tag="recip")
                        nc.vector.reciprocal(recip[:], sums_p[:])
                        recip_r = sbmisc.tile([1, SBLK], F32R, tag="recipr")
                        nc.vector.tensor_copy(recip_r[:], recip[:])
                        rep_p = psr.tile([128, SBLK], F32, tag="rep")
                        nc.tensor.matmul(rep_p[:], ones1_r[:], recip_r[:],
                                         start=True, stop=True)
                        rep_s = sbmisc.tile([128, SBLK], F32, tag="reps")
                        nc.vector.tensor_copy(rep_s[:], rep_p[:])
                        ctx_s = sbmisc.tile([128, SBLK], F32, tag="ctxs")
                        nc.vector.tensor_mul(ctx_s[:], ctx_p[:], rep_s[:])
                        nc.sync.dma_start(out=ctxT_d[h, :, s0:s0 + SBLK],
                                          in_=ctx_s[:])

            if repeat == 1:
                body()
            else:
                with tc.For_i(0, repeat, 1):
                    body()
    nc.compile()
    return nc


def _get_nc(repeat=1):
    if repeat not in _CACHE:
        _CACHE[repeat] = _build(repeat)
    return _CACHE[repeat]


def _make_in_maps(query_layer, key_layer, value_layer, attention_mask):
    q = np.asarray(query_layer, dtype=np.float32)
    k = np.asarray(key_layer, dtype=np.float32)
    v = np.asarray(value_layer, dtype=np.float32)
    m = np.asarray(attention_mask)
    in_maps = []
    for c in range(NCORES):
        b = c // 4
        h0 = 4 * (c % 4)
        hs = slice(h0, h0 + HPC)
        qT = np.ascontiguousarray(q[:, b, hs, :].transpose(1, 2, 0))  # [4,hn,sq]
        kT = np.ascontiguousarray(k[:, b, hs, :].transpose(1, 2, 0))
        vv = np.ascontiguousarray(v[:, b, hs, :].transpose(1, 0, 2))  # [4,sq,hn]
        keepT = np.ascontiguousarray(
            (m[b, 0] == 0).astype(np.uint8).T)                        # [t,s] u8
        in_maps.append({"qT": qT, "kT": kT, "v": vv, "keepT": keepT})
    return in_maps


def _run(nc, in_maps):
    from concourse.bass_utils import run_bass_kernel_spmd
    return run_bass_kernel_spmd(nc, in_maps, list(range(NCORES)))


def kernel(query_layer, key_layer, value_layer, attention_mask):
    in_maps = _make_in_maps(query_layer, key_layer, value_layer, attention_mask)
    res = _run(_get_nc(1), in_maps)
    out = np.empty((SQ, B, NH, HN), dtype=np.float32)
    for c in range(NCORES):
        b = c // 4
        h0 = 4 * (c % 4)
        ctxT = res.results[c]["ctxT"]                                 # [4,hn,sq]
        out[:, b, h0:h0 + HPC, :] = ctxT.transpose(2, 0, 1)
    return out.reshape(SQ, B, NH * HN)



# revision 3
# speedup vs baseline: 6.0089x; 6.0089x over previous
"""CoreAttention on 8 Trainium2 cores — bf16 pipeline, wide ACT calls.

Sharding: 32 (batch, head) pairs -> 4 heads per core (cores 0-3: batch 0,
cores 4-7: batch 1). Per core, per head, per 512-wide query block:
scores^T = K Q^T in [t, s] orientation via bf16 matmuls (fp32 PSUM) in
3-chunk groups so each ACT exp call covers 1536 elements/partition (ACT is
per-call-overhead bound at ~1.2us/call, so wider calls matter more than
anything else). exp on ACT (softmax scale fused, bf16 out), mask applied as
a bf16 multiply on DVE, softmax denominator via one strided DVE pair-add +
8 ones-matmuls into PSUM, normalization via reciprocal + rank-1 broadcast
matmul (f32r). PSUM budget: score groups 3 banks x2 bufs, ctx 1, sums/rep
share 1 bank via tag. Per-head staged f32 output, one contiguous DMA per
head. Host side only slices/transposes/casts inputs (contiguous layouts so
every DMA is 128 whole-partition descriptors).
"""
import math
import numpy as np
import ml_dtypes

SQ, B, NH, HN = 2048, 2, 16, 128
NCORES = 8
HPC = 4                      # heads per core
TC = SQ // 128               # 16 t-chunks
SBLK = 512                   # s-block width
NSB = SQ // SBLK             # 4 s-blocks
SCALE = 1.0 / math.sqrt(128.0)   # COEFF / NORM_FACTOR = 1/sqrt(hn)

_CACHE = {}


def _build(repeat=1):
    import concourse.bacc as bacc
    import concourse.tile as tile
    from concourse import mybir

    F32, F32R = mybir.dt.float32, mybir.dt.float32r
    BF16 = mybir.dt.bfloat16
    EXP = mybir.ActivationFunctionType.Exp

    nc = bacc.Bacc(None, target_bir_lowering=False)
    qT_d = nc.dram_tensor("qTh", [HPC, HN, SQ], BF16, kind="ExternalInput")
    kT_d = nc.dram_tensor("kTh", [HPC, HN, SQ], BF16, kind="ExternalInput")
    v_d = nc.dram_tensor("vp", [HPC, 128, TC * HN], BF16, kind="ExternalInput")
    keep_d = nc.dram_tensor("keepp", [128, TC * SQ], BF16,
                            kind="ExternalInput")
    ctxT_d = nc.dram_tensor("ctxT", [HPC, HN, SQ], F32, kind="ExternalOutput")

    GROUPS = [(0, 3), (3, 3), (6, 3), (9, 3), (12, 3), (15, 1)]

    with tile.TileContext(nc) as tc:
        with (
            tc.tile_pool(name="sbkeep", bufs=1) as sbkeep,
            tc.tile_pool(name="const", bufs=1) as const,
            tc.tile_pool(name="sbqkv", bufs=2) as sbqkv,
            tc.tile_pool(name="sbpt", bufs=2) as sbpt,
            tc.tile_pool(name="sbe", bufs=3) as sbe,
            tc.tile_pool(name="sbmisc", bufs=2) as sbmisc,
            tc.tile_pool(name="sbout", bufs=2) as sbout,
            tc.tile_pool(name="pst", bufs=2, space="PSUM") as pst,
            tc.tile_pool(name="psc", bufs=1, space="PSUM") as psc,
            tc.tile_pool(name="pss", bufs=1, space="PSUM") as pss,
        ):
            keep_t = sbkeep.tile([128, TC, SQ], BF16, tag="keep")
            nc.sync.dma_start(out=keep_t[:],
                              in_=keep_d.rearrange("p (c s) -> p c s", c=TC))

            ones32 = const.tile([128, 1], F32, tag="o32")
            ones1_32 = const.tile([1, 128], F32, tag="o132")
            nc.vector.memset(ones32[:], 1.0)
            nc.vector.memset(ones1_32[:], 1.0)
            ones_h = const.tile([128, 1], BF16, tag="oh")
            ones1_r = const.tile([1, 128], F32R, tag="o1r")
            nc.vector.tensor_copy(ones_h[:], ones32[:])
            nc.vector.tensor_copy(ones1_r[:], ones1_32[:])

            def body(_iv=None):
                for h in range(HPC):
                    qT_t = sbqkv.tile([128, SQ], BF16, tag="qT")
                    kT_t = sbqkv.tile([128, SQ], BF16, tag="kT")
                    v_t = sbqkv.tile([128, TC, HN], BF16, tag="v")
                    nc.sync.dma_start(out=qT_t[:], in_=qT_d[h])
                    nc.sync.dma_start(out=kT_t[:], in_=kT_d[h])
                    nc.sync.dma_start(
                        out=v_t[:],
                        in_=v_d[h].rearrange("p (c d) -> p c d", c=TC))
                    stage = sbout.tile([128, SQ], F32, tag="stage")
                    for sb in range(NSB):
                        s0 = sb * SBLK
                        pt = sbpt.tile([128, TC, SBLK], BF16, tag="pt")
                        for g0, gw in GROUPS:
                            st = pst.tile([128, 3, SBLK], F32, tag="st")
                            for j in range(gw):
                                ti = g0 + j
                                nc.tensor.matmul(
                                    st[:, j, :],
                                    kT_t[:, 128 * ti:128 * (ti + 1)],
                                    qT_t[:, s0:s0 + SBLK],
                                    start=True, stop=True)
                            e_t = sbe.tile([128, 3, SBLK], BF16, tag="e")
                            nc.scalar.activation(e_t[:, 0:gw, :],
                                                 st[:, 0:gw, :], EXP,
                                                 scale=SCALE)
                            nc.vector.tensor_mul(
                                pt[:, g0:g0 + gw, :], e_t[:, 0:gw, :],
                                keep_t[:, g0:g0 + gw, s0:s0 + SBLK])
                        prs = sbe.tile([128, 8, SBLK], BF16, tag="prs")
                        ptv = pt[:].rearrange("p (a b) s -> p a b s", b=2)
                        nc.vector.tensor_add(prs[:], ptv[:, :, 0, :],
                                             ptv[:, :, 1, :])
                        ctx_p = psc.tile([128, SBLK], F32, tag="ctx")
                        sums_p = pss.tile([1, SBLK], F32, tag="sr")
                        for ti in range(TC):
                            nc.tensor.matmul(ctx_p[:], v_t[:, ti, :],
                                             pt[:, ti, :],
                                             start=(ti == 0),
                                             stop=(ti == TC - 1))
                        for j in range(8):
                            nc.tensor.matmul(sums_p[:], ones_h[:],
                                             prs[:, j, :],
                                             start=(j == 0), stop=(j == 7))
                        recip = sbmisc.tile([1, SBLK], F32, tag="recip")
                        nc.vector.reciprocal(recip[:], sums_p[:])
                        recip_r = sbmisc.tile([1, SBLK], F32R, tag="recipr")
                        nc.vector.tensor_copy(recip_r[:], recip[:])
                        rep_p = pss.tile([128, SBLK], F32, tag="sr")
                        nc.tensor.matmul(rep_p[:], ones1_r[:], recip_r[:],
                                         start=True, stop=True)
                        rep_s = sbmisc.tile([128, SBLK], F32, tag="reps")
                        nc.vector.tensor_copy(rep_s[:], rep_p[:])
                        nc.vector.tensor_mul(stage[:, s0:s0 + SBLK], ctx_p[:],
                                             rep_s[:])
                    nc.sync.dma_start(out=ctxT_d[h], in_=stage[:])

            if repeat == 1:
                body()
            else:
                with tc.For_i(0, repeat, 1):
                    body()
    nc.compile()
    return nc


def _get_nc(repeat=1):
    if repeat not in _CACHE:
        _CACHE[repeat] = _build(repeat)
    return _CACHE[repeat]


def _make_in_maps(query_layer, key_layer, value_layer, attention_mask):
    bf16 = ml_dtypes.bfloat16
    q = np.asarray(query_layer, dtype=np.float32)
    k = np.asarray(key_layer, dtype=np.float32)
    v = np.asarray(value_layer, dtype=np.float32)
    m = np.asarray(attention_mask)
    in_maps = []
    for c in range(NCORES):
        b = c // 4
        h0 = 4 * (c % 4)
        hs = slice(h0, h0 + HPC)
        qT = np.ascontiguousarray(
            q[:, b, hs, :].transpose(1, 2, 0)).astype(bf16)    # [4,hn,sq]
        kT = np.ascontiguousarray(
            k[:, b, hs, :].transpose(1, 2, 0)).astype(bf16)
        vv = v[:, b, hs, :].transpose(1, 0, 2)                 # [4,sq,hn]
        vp = np.ascontiguousarray(
            vv.reshape(HPC, TC, 128, HN).transpose(0, 2, 1, 3)
            .reshape(HPC, 128, TC * HN)).astype(bf16)
        keepT = (m[b, 0] == 0).astype(np.float32).T            # [t, s]
        keepp = np.ascontiguousarray(
            keepT.reshape(TC, 128, SQ).transpose(1, 0, 2)
            .reshape(128, TC * SQ)).astype(bf16)
        in_maps.append({"qTh": qT, "kTh": kT, "vp": vp, "keepp": keepp})
    return in_maps


def _run(nc, in_maps):
    from concourse.bass_utils import run_bass_kernel_spmd
    return run_bass_kernel_spmd(nc, in_maps, list(range(NCORES)))


def kernel(query_layer, key_layer, value_layer, attention_mask):
    in_maps = _make_in_maps(query_layer, key_layer, value_layer,
                            attention_mask)
    res = _run(_get_nc(1), in_maps)
    out = np.empty((SQ, B, NH, HN), dtype=np.float32)
    for c in range(NCORES):
        b = c // 4
        h0 = 4 * (c % 4)
        ctxT = res.results[c]["ctxT"]                          # [4,hn,sq]
        out[:, b, h0:h0 + HPC, :] = ctxT.transpose(2, 0, 1)
    return out.reshape(SQ, B, NH * HN)
